# revision 25
# baseline (speedup 1.0000x reference)
"""MoE feed-forward (8 experts, top-2) on 8 Trainium2 NeuronCores.

Expert-parallel: core e holds expert e's weights. The (cheap) router runs on
host; tokens are dispatched to expert cores host-side, each core runs
  y = cw * (gelu(x @ W1 + b1) @ W2 + b2)
for its tokens, and the host combines the per-expert contributions back
into token order.

Device numerics: matmul operands in fp16 (PSUM accumulation fp32, GELU/bias/
combine-weights fp32) -> ~6e-4 relative error overall.

Fixed problem shape (hardcoded per contest contract):
  x [4, 2048, 1024], Wr [8, 1024], W1 [8, 1024, 4096], b1 [8, 4096],
  W2 [8, 4096, 1024], b2 [8, 1024]. TOP_K = 2.
"""

import math
import os

import numpy as np

import concourse.bass as bass
import concourse.mybir as mybir
import concourse.tile as tile
from concourse import bacc
from concourse.bass import ts
from concourse.bass_utils import run_bass_kernel_spmd

D = 1024  # d_model
F = 4096  # ff dim
E = 8  # experts == cores
TOP_K = 2
CAP = 2112  # tokens per expert-core (seed-0 max load 2078; overflow -> host)
# token blocks (moving free dim); small first block lets PE start early,
# 512 in steady state keeps the per-matmul weight load hidden
BLOCKS = [(0, 128), (128, 384), (512, 512), (1024, 512), (1536, 512), (2048, 64)]
BLK = 512
KD = D // 128  # 8   contraction tiles for GEMM1
KF = F // 128  # 32  contraction tiles for GEMM2
MF = F // 128  # 32  output tiles for GEMM1
MD = D // 128  # 8   output tiles for GEMM2

F32 = mybir.dt.float32
F16 = mybir.dt.float16

# Cache the built+finalized Bass graph across kernel() calls in one process.
_NC_CACHE = {}

LAST_RESULT = None  # BassKernelResults of the most recent device run


def _build_nc():
    nc = bacc.Bacc("TRN2", target_bir_lowering=False)

    xT = nc.declare_dram_parameter("xT", [128, KD, CAP], F16, isOutput=False)
    # weights pre-chunked on host: w1 grouped 4 m-slices per contiguous chunk
    w1 = nc.declare_dram_parameter("w1", [MF // 4, 128, 4, KD, 128], F16, isOutput=False)
    b1c = nc.declare_dram_parameter("b1c", [128, MF], F32, isOutput=False)
    w2 = nc.declare_dram_parameter("w2", [MD // 4, 128, 4, KF, 128], F16, isOutput=False)
    b2c = nc.declare_dram_parameter("b2c", [128, MD], F32, isOutput=False)
    cwb = nc.declare_dram_parameter("cwb", [128, CAP], F32, isOutput=False)
    yT = nc.declare_dram_parameter("yT", [128, MD, CAP], F32, isOutput=True)

    with tile.TileContext(nc) as tc:
        with (
            tc.tile_pool(name="w1p", bufs=1) as w1pool,
            tc.tile_pool(name="w2p", bufs=1) as w2pool,
            tc.tile_pool(name="hp", bufs=1) as hpool,
            tc.tile_pool(name="xbp", bufs=4) as xbpool,
            tc.tile_pool(name="stage", bufs=3) as spool,
            tc.tile_pool(name="cwblk", bufs=2) as cbpool,
            tc.tile_pool(name="const", bufs=1) as cpool,
            tc.tile_pool(name="ps", bufs=4, space="PSUM") as pspool,
        ):
            # first token block loads before everything so PE starts early
            xb0 = xbpool.tile([128, KD, BLK], F16, tag="xb")
            nc.sync.dma_start(out=xb0[:, :, : BLOCKS[0][1]], in_=xT[:, :, 0 : BLOCKS[0][1]])

            b1t = cpool.tile([128, MF], F32)
            nc.sync.dma_start(out=b1t[:], in_=b1c[:])
            b2t = cpool.tile([128, MD], F32)
            nc.sync.dma_start(out=b2t[:], in_=b2c[:])

            # w1: grouped m-slice loads (matmuls for tile m wait only for its group)
            w1t = w1pool.tile([128, MF, KD, 128], F16, tag="w1")
            # slice 0 alone so the first matmul's dependency is tiny
            nc.sync.dma_start(
                out=w1t[:, 0].rearrange("p k q -> p (k q)"),
                in_=w1[0, :, 0].rearrange("p k q -> p (k q)"),
            )
            nc.sync.dma_start(
                out=w1t[:, 1:4].rearrange("p m k q -> p (m k q)"),
                in_=w1[0, :, 1:4].rearrange("p m k q -> p (m k q)"),
            )
            for g in range(1, MF // 4):
                nc.sync.dma_start(
                    out=w1t[:, ts(g, 4)].rearrange("p m k q -> p (m k q)"),
                    in_=w1[g].rearrange("p m k q -> p (m k q)"),
                )
            # w2 slices are issued below, spread across early blocks
            w2t = w2pool.tile([128, MD, KF, 128], F16, tag="w2")

            for bi, (off, bw) in enumerate(BLOCKS):
                if off == 0:
                    xb = xb0
                else:
                    xb = xbpool.tile([128, KD, BLK], F16, tag="xb")
                    nc.sync.dma_start(
                        out=xb[:, :, :bw], in_=xT[:, :, off : off + bw]
                    )
                cb = cbpool.tile([128, BLK], F32, tag="cb")
                nc.sync.dma_start(out=cb[:, :bw], in_=cwb[:, off : off + bw])

                # ---- GEMM1: h = gelu(W1.T @ x + b1), h stays in SBUF ----
                h = hpool.tile([128, MF, BLK], F16, tag="h")
                for m in range(MF):
                    if bi == 0 and m % 4 == 0:
                        # spread w2 slice loads across block 0's GEMM1 so they
                        # are resident before its GEMM2 without starving the
                        # just-in-time w1/xb deliveries
                        mw = m // 4
                        g, r = divmod(mw, 4)
                        nc.sync.dma_start(
                            out=w2t[:, mw].rearrange("p k q -> p (k q)"),
                            in_=w2[g, :, r].rearrange("p k q -> p (k q)"),
                        )
                    ps = pspool.tile([128, BLK], F32, tag="ps")
                    for k in range(KD):
                        nc.tensor.matmul(
                            ps[:, :bw],
                            lhsT=w1t[:, m, k],
                            rhs=xb[:, k, :bw],
                            start=(k == 0),
                            stop=(k == KD - 1),
                        )
                    nc.scalar.activation(
                        h[:, m, :bw],
                        ps[:, :bw],
                        mybir.ActivationFunctionType.Gelu,
                        bias=b1t[:, m : m + 1],
                    )

                # ---- GEMM2: yT = cw * (W2.T @ h + b2) ----
                for m in range(MD):
                    ps = pspool.tile([128, BLK], F32, tag="ps")
                    for k in range(KF):
                        nc.tensor.matmul(
                            ps[:, :bw],
                            lhsT=w2t[:, m, k],
                            rhs=h[:, k, :bw],
                            start=(k == 0),
                            stop=(k == KF - 1),
                        )
                    ys = spool.tile([128, BLK], F32, tag="ys")
                    nc.scalar.activation(
                        ys[:, :bw],
                        ps[:, :bw],
                        mybir.ActivationFunctionType.Identity,
                        bias=b2t[:, m : m + 1],
                    )
                    nc.vector.tensor_mul(
                        out=ys[:, :bw], in0=ys[:, :bw], in1=cb[:, :bw]
                    )
                    nc.sync.dma_start(out=yT[:, m, off : off + bw], in_=ys[:, :bw])

    nc.finalize()
    return nc


def _gelu_exact_np(x):
    try:
        from scipy.special import erf

        return 0.5 * x * (1.0 + erf(x / np.sqrt(2.0)))
    except ImportError:
        _erf = np.vectorize(math.erf)
        return 0.5 * x * (1.0 + _erf(x / np.sqrt(2.0)))


def _route(t, Wr):
    """Replicate the reference router in fp32 numpy: softmax + top-2 with
    jax.lax.top_k tie-breaking (first index wins), weights renormalized."""
    logits = t @ Wr.T  # [T, E] fp32
    mx = logits.max(axis=1, keepdims=True)
    ez = np.exp(logits - mx, dtype=np.float32)
    probs = ez / ez.sum(axis=1, keepdims=True, dtype=np.float32)

    arange = np.arange(t.shape[0])
    i1 = probs.argmax(axis=1)
    masked = probs.copy()
    masked[arange, i1] = -np.inf
    i2 = masked.argmax(axis=1)
    v1 = probs[arange, i1]
    v2 = probs[arange, i2]
    s = v1 + v2
    return i1, i2, v1 / s, v2 / s


def kernel(x, Wr, W1, b1, W2, b2):
    global LAST_RESULT

    x = np.asarray(x, dtype=np.float32)
    Wr = np.asarray(Wr, dtype=np.float32)
    W1 = np.asarray(W1, dtype=np.float32)
    b1 = np.asarray(b1, dtype=np.float32)
    W2 = np.asarray(W2, dtype=np.float32)
    b2 = np.asarray(b2, dtype=np.float32)

    Bb, Ss, _ = x.shape
    T = Bb * Ss
    t = np.ascontiguousarray(x.reshape(T, D))

    i1, i2, cw1, cw2 = _route(t, Wr)

    # per-expert token lists (device handles first CAP; remainder -> host)
    dev_idx, dev_cw, host_idx, host_cw = [], [], [], []
    for e in range(E):
        sel1 = np.nonzero(i1 == e)[0]
        sel2 = np.nonzero(i2 == e)[0]
        idx = np.concatenate([sel1, sel2])
        cw = np.concatenate([cw1[sel1], cw2[sel2]]).astype(np.float32)
        dev_idx.append(idx[:CAP])
        dev_cw.append(cw[:CAP])
        host_idx.append(idx[CAP:])
        host_cw.append(cw[CAP:])

    in_maps = []
    for e in range(E):
        idx = dev_idx[e]
        n = len(idx)
        xe = np.zeros((128, KD, CAP), dtype=np.float16)
        # t[idx] : [n, D] -> [n, KD, 128] -> [128, KD, n]
        xe[:, :, :n] = t[idx].reshape(n, KD, 128).transpose(2, 1, 0)
        cwe = np.zeros((CAP,), dtype=np.float32)
        cwe[:n] = dev_cw[e]
        # W1[e]: [D, F] -> [m][p][k][q] with row index k*128+p, col index m*128+q
        # [K,128p,G,4m,128q] -> [G, p, m, K, q]
        w1e = np.ascontiguousarray(
            W1[e].reshape(KD, 128, MF // 4, 4, 128).transpose(2, 1, 3, 0, 4),
            dtype=np.float16,
        )
        w2e = np.ascontiguousarray(
            W2[e].reshape(KF, 128, MD // 4, 4, 128).transpose(2, 1, 3, 0, 4),
            dtype=np.float16,
        )
        in_maps.append(
            {
                "xT": xe,
                "w1": w1e,
                "b1c": np.ascontiguousarray(b1[e].reshape(MF, 128).T),
                "w2": w2e,
                "b2c": np.ascontiguousarray(b2[e].reshape(MD, 128).T),
                "cwb": np.broadcast_to(cwe, (128, CAP)).copy(),
            }
        )

    if "nc" not in _NC_CACHE:
        _NC_CACHE["nc"] = _build_nc()
    nc = _NC_CACHE["nc"]

    try:
        res = run_bass_kernel_spmd(nc, in_maps, core_ids=list(range(E)))
    except ModuleNotFoundError:
        # BASS_TRACE was requested but this environment lacks the axon NTFF
        # profiling hook module; rerun with tracing disabled
        os.environ["BASS_NEVER_TRACE"] = "1"
        res = run_bass_kernel_spmd(nc, in_maps, core_ids=list(range(E)))
    LAST_RESULT = res

    out = np.zeros((T, D), dtype=np.float32)
    for e in range(E):
        idx = dev_idx[e]
        n = len(idx)
        if n == 0:
            continue
        yT = res.results[e]["yT"]  # [128, MD, CAP]
        ye = yT.transpose(2, 1, 0).reshape(CAP, D)[:n]  # [n, D]
        out[idx] += ye

    # exact host fallback for (rare) capacity overflow
    for e in range(E):
        idx = host_idx[e]
        if len(idx) == 0:
            continue
        h = _gelu_exact_np(t[idx] @ W1[e] + b1[e]).astype(np.float32)
        ye = (h @ W2[e] + b2[e]) * host_cw[e][:, None]
        out[idx] += ye.astype(np.float32)

    return out.reshape(Bb, Ss, D)


# revision 26
# speedup vs baseline: 1.0425x; 1.0425x over previous
"""MoE feed-forward (8 experts, top-2) on 8 Trainium2 NeuronCores.

Expert-parallel: core e holds expert e's weights. The (cheap) router runs on
host; tokens are dispatched to expert cores host-side, each core runs
  y = cw * (gelu(x @ W1 + b1) @ W2 + b2)
for its tokens, and the host combines the per-expert contributions back
into token order.

Device numerics: matmul operands in fp16 (PSUM accumulation fp32, GELU/bias/
combine-weights fp32) -> ~6e-4 relative error overall.

Fixed problem shape (hardcoded per contest contract):
  x [4, 2048, 1024], Wr [8, 1024], W1 [8, 1024, 4096], b1 [8, 4096],
  W2 [8, 4096, 1024], b2 [8, 1024]. TOP_K = 2.
"""

import math
import os

import numpy as np

import concourse.bass as bass
import concourse.mybir as mybir
import concourse.tile as tile
from concourse import bacc
from concourse.bass import ts
from concourse.bass_utils import run_bass_kernel_spmd

D = 1024  # d_model
F = 4096  # ff dim
E = 8  # experts == cores
TOP_K = 2
CAP = 2112  # tokens per expert-core (seed-0 max load 2078; overflow -> host)
# token blocks (moving free dim); small first block lets PE start early,
# 512 in steady state keeps the per-matmul weight load hidden
BLOCKS = [(0, 256), (256, 512), (768, 512), (1280, 512), (1792, 320)]
BLK = 512
KD = D // 128  # 8   contraction tiles for GEMM1
KF = F // 128  # 32  contraction tiles for GEMM2
MF = F // 128  # 32  output tiles for GEMM1
MD = D // 128  # 8   output tiles for GEMM2

F32 = mybir.dt.float32
F16 = mybir.dt.float16

# Cache the built+finalized Bass graph across kernel() calls in one process.
_NC_CACHE = {}

LAST_RESULT = None  # BassKernelResults of the most recent device run


def _build_nc():
    nc = bacc.Bacc("TRN2", target_bir_lowering=False)

    xT = nc.declare_dram_parameter("xT", [128, KD, CAP], F16, isOutput=False)
    # weights pre-chunked on host: w1 grouped 4 m-slices per contiguous chunk
    w1 = nc.declare_dram_parameter("w1", [MF // 4, 128, 4, KD, 128], F16, isOutput=False)
    b1c = nc.declare_dram_parameter("b1c", [128, MF], F32, isOutput=False)
    w2 = nc.declare_dram_parameter("w2", [MD // 4, 128, 4, KF, 128], F16, isOutput=False)
    b2c = nc.declare_dram_parameter("b2c", [128, MD], F32, isOutput=False)
    cwb = nc.declare_dram_parameter("cwb", [128, CAP], F32, isOutput=False)
    yT = nc.declare_dram_parameter("yT", [128, MD, CAP], F32, isOutput=True)

    with tile.TileContext(nc) as tc:
        with (
            tc.tile_pool(name="w1p", bufs=1) as w1pool,
            tc.tile_pool(name="w2p", bufs=1) as w2pool,
            tc.tile_pool(name="hp", bufs=1) as hpool,
            tc.tile_pool(name="xbp", bufs=4) as xbpool,
            tc.tile_pool(name="stage", bufs=3) as spool,
            tc.tile_pool(name="cwblk", bufs=2) as cbpool,
            tc.tile_pool(name="const", bufs=1) as cpool,
            tc.tile_pool(name="ps", bufs=4, space="PSUM") as pspool,
        ):
            # first token block loads before everything so PE starts early
            xb0 = xbpool.tile([128, KD, BLK], F16, tag="xb")
            nc.sync.dma_start(out=xb0[:, :, : BLOCKS[0][1]], in_=xT[:, :, 0 : BLOCKS[0][1]])

            b1t = cpool.tile([128, MF], F32)
            nc.sync.dma_start(out=b1t[:], in_=b1c[:])
            b2t = cpool.tile([128, MD], F32)
            nc.sync.dma_start(out=b2t[:], in_=b2c[:])

            # w1: grouped m-slice loads (matmuls for tile m wait only for its group)
            w1t = w1pool.tile([128, MF, KD, 128], F16, tag="w1")
            # slice 0 alone so the first matmul's dependency is tiny
            nc.sync.dma_start(
                out=w1t[:, 0].rearrange("p k q -> p (k q)"),
                in_=w1[0, :, 0].rearrange("p k q -> p (k q)"),
            )
            nc.sync.dma_start(
                out=w1t[:, 1:4].rearrange("p m k q -> p (m k q)"),
                in_=w1[0, :, 1:4].rearrange("p m k q -> p (m k q)"),
            )
            for g in range(1, MF // 4):
                nc.sync.dma_start(
                    out=w1t[:, ts(g, 4)].rearrange("p m k q -> p (m k q)"),
                    in_=w1[g].rearrange("p m k q -> p (m k q)"),
                )
            # w2 slices are issued below, spread across early blocks
            w2t = w2pool.tile([128, MD, KF, 128], F16, tag="w2")

            for bi, (off, bw) in enumerate(BLOCKS):
                if off == 0:
                    xb = xb0
                else:
                    xb = xbpool.tile([128, KD, BLK], F16, tag="xb")
                    nc.sync.dma_start(
                        out=xb[:, :, :bw], in_=xT[:, :, off : off + bw]
                    )
                cb = cbpool.tile([128, BLK], F32, tag="cb")
                nc.sync.dma_start(out=cb[:, :bw], in_=cwb[:, off : off + bw])

                # ---- GEMM1: h = gelu(W1.T @ x + b1), h stays in SBUF ----
                h = hpool.tile([128, MF, BLK], F16, tag="h")
                for m in range(MF):
                    if bi == 0 and m % 4 == 0:
                        # spread w2 slice loads across block 0's GEMM1 so they
                        # are resident before its GEMM2 without starving the
                        # just-in-time w1/xb deliveries
                        mw = m // 4
                        g, r = divmod(mw, 4)
                        nc.sync.dma_start(
                            out=w2t[:, mw].rearrange("p k q -> p (k q)"),
                            in_=w2[g, :, r].rearrange("p k q -> p (k q)"),
                        )
                    ps = pspool.tile([128, BLK], F32, tag="ps")
                    for k in range(KD):
                        nc.tensor.matmul(
                            ps[:, :bw],
                            lhsT=w1t[:, m, k],
                            rhs=xb[:, k, :bw],
                            start=(k == 0),
                            stop=(k == KD - 1),
                        )
                    nc.scalar.activation(
                        h[:, m, :bw],
                        ps[:, :bw],
                        mybir.ActivationFunctionType.Gelu,
                        bias=b1t[:, m : m + 1],
                    )

                # ---- GEMM2: yT = cw * (W2.T @ h + b2) ----
                for m in range(MD):
                    ps = pspool.tile([128, BLK], F32, tag="ps")
                    for k in range(KF):
                        nc.tensor.matmul(
                            ps[:, :bw],
                            lhsT=w2t[:, m, k],
                            rhs=h[:, k, :bw],
                            start=(k == 0),
                            stop=(k == KF - 1),
                        )
                    ys = spool.tile([128, BLK], F32, tag="ys")
                    nc.scalar.activation(
                        ys[:, :bw],
                        ps[:, :bw],
                        mybir.ActivationFunctionType.Identity,
                        bias=b2t[:, m : m + 1],
                    )
                    nc.vector.tensor_mul(
                        out=ys[:, :bw], in0=ys[:, :bw], in1=cb[:, :bw]
                    )
                    nc.sync.dma_start(out=yT[:, m, off : off + bw], in_=ys[:, :bw])

    nc.finalize()
    return nc


def _gelu_exact_np(x):
    try:
        from scipy.special import erf

        return 0.5 * x * (1.0 + erf(x / np.sqrt(2.0)))
    except ImportError:
        _erf = np.vectorize(math.erf)
        return 0.5 * x * (1.0 + _erf(x / np.sqrt(2.0)))


def _route(t, Wr):
    """Replicate the reference router in fp32 numpy: softmax + top-2 with
    jax.lax.top_k tie-breaking (first index wins), weights renormalized."""
    logits = t @ Wr.T  # [T, E] fp32
    mx = logits.max(axis=1, keepdims=True)
    ez = np.exp(logits - mx, dtype=np.float32)
    probs = ez / ez.sum(axis=1, keepdims=True, dtype=np.float32)

    arange = np.arange(t.shape[0])
    i1 = probs.argmax(axis=1)
    masked = probs.copy()
    masked[arange, i1] = -np.inf
    i2 = masked.argmax(axis=1)
    v1 = probs[arange, i1]
    v2 = probs[arange, i2]
    s = v1 + v2
    return i1, i2, v1 / s, v2 / s


def kernel(x, Wr, W1, b1, W2, b2):
    global LAST_RESULT

    x = np.asarray(x, dtype=np.float32)
    Wr = np.asarray(Wr, dtype=np.float32)
    W1 = np.asarray(W1, dtype=np.float32)
    b1 = np.asarray(b1, dtype=np.float32)
    W2 = np.asarray(W2, dtype=np.float32)
    b2 = np.asarray(b2, dtype=np.float32)

    Bb, Ss, _ = x.shape
    T = Bb * Ss
    t = np.ascontiguousarray(x.reshape(T, D))

    i1, i2, cw1, cw2 = _route(t, Wr)

    # per-expert token lists (device handles first CAP; remainder -> host)
    dev_idx, dev_cw, host_idx, host_cw = [], [], [], []
    for e in range(E):
        sel1 = np.nonzero(i1 == e)[0]
        sel2 = np.nonzero(i2 == e)[0]
        idx = np.concatenate([sel1, sel2])
        cw = np.concatenate([cw1[sel1], cw2[sel2]]).astype(np.float32)
        dev_idx.append(idx[:CAP])
        dev_cw.append(cw[:CAP])
        host_idx.append(idx[CAP:])
        host_cw.append(cw[CAP:])

    in_maps = []
    for e in range(E):
        idx = dev_idx[e]
        n = len(idx)
        xe = np.zeros((128, KD, CAP), dtype=np.float16)
        # t[idx] : [n, D] -> [n, KD, 128] -> [128, KD, n]
        xe[:, :, :n] = t[idx].reshape(n, KD, 128).transpose(2, 1, 0)
        cwe = np.zeros((CAP,), dtype=np.float32)
        cwe[:n] = dev_cw[e]
        # W1[e]: [D, F] -> [m][p][k][q] with row index k*128+p, col index m*128+q
        # [K,128p,G,4m,128q] -> [G, p, m, K, q]
        w1e = np.ascontiguousarray(
            W1[e].reshape(KD, 128, MF // 4, 4, 128).transpose(2, 1, 3, 0, 4),
            dtype=np.float16,
        )
        w2e = np.ascontiguousarray(
            W2[e].reshape(KF, 128, MD // 4, 4, 128).transpose(2, 1, 3, 0, 4),
            dtype=np.float16,
        )
        in_maps.append(
            {
                "xT": xe,
                "w1": w1e,
                "b1c": np.ascontiguousarray(b1[e].reshape(MF, 128).T),
                "w2": w2e,
                "b2c": np.ascontiguousarray(b2[e].reshape(MD, 128).T),
                "cwb": np.broadcast_to(cwe, (128, CAP)).copy(),
            }
        )

    if "nc" not in _NC_CACHE:
        _NC_CACHE["nc"] = _build_nc()
    nc = _NC_CACHE["nc"]

    try:
        res = run_bass_kernel_spmd(nc, in_maps, core_ids=list(range(E)))
    except ModuleNotFoundError:
        # BASS_TRACE was requested but this environment lacks the axon NTFF
        # profiling hook module; rerun with tracing disabled
        os.environ["BASS_NEVER_TRACE"] = "1"
        res = run_bass_kernel_spmd(nc, in_maps, core_ids=list(range(E)))
    LAST_RESULT = res

    out = np.zeros((T, D), dtype=np.float32)
    for e in range(E):
        idx = dev_idx[e]
        n = len(idx)
        if n == 0:
            continue
        yT = res.results[e]["yT"]  # [128, MD, CAP]
        ye = yT.transpose(2, 1, 0).reshape(CAP, D)[:n]  # [n, D]
        out[idx] += ye

    # exact host fallback for (rare) capacity overflow
    for e in range(E):
        idx = host_idx[e]
        if len(idx) == 0:
            continue
        h = _gelu_exact_np(t[idx] @ W1[e] + b1[e]).astype(np.float32)
        ye = (h @ W2[e] + b2[e]) * host_cw[e][:, None]
        out[idx] += ye.astype(np.float32)

    return out.reshape(Bb, Ss, D)
